# revision 18
# baseline (speedup 1.0000x reference)
"""Trainium2 Bass kernel for the DanceDynamicsModel Lindblad solver.

Full inputs in, full outputs out. Internally:
  - host (numpy): build the 128x128 Hamiltonian H, the 49 Lindblad
    operators L_k, and M = sum_k L_k^T L_k from the tiny MLP inputs.
  - device (8 NeuronCores): 4 RK4 steps of the linear Lindblad ODE in
    Taylor form (v_{j+1} = F(v_j), rho' = rho + sum_j c_j v_j), with the
    49 L-sandwich terms sharded over cores and an AllReduce per stage.

All device matmuls are bf16 (PSUM accumulates fp32); validated to give
~1e-6 global relative error vs the complex64 reference.
"""
import sys
for _p in ('/opt/trn_rl_repo',):
    if _p not in sys.path:
        sys.path.insert(0, _p)

import numpy as np
import ml_dtypes

import concourse.bass as bass
import concourse.bacc as bacc
import concourse.tile as tile
import concourse.mybir as mybir

NQ = 7          # qubits ("dancers")
D = 128         # 2**NQ
NCORES = 8
SLOTS = 7       # Lindblad-op slots per core (49 real ops + AB task, padded)
# ops per core; core 0 also owns the Hamiltonian/M ("AB") terms
OP_SPLIT = [4, 7, 7, 7, 6, 6, 6, 6]
BF16 = mybir.dt.bfloat16
F32 = mybir.dt.float32
AluOp = mybir.AluOpType


# ----------------------------------------------------------------- host math
def _embed(op, sites):
    k = len(sites)
    full = np.kron(op, np.eye(2 ** (NQ - k), dtype=op.dtype))
    t = full.reshape((2,) * (2 * NQ))
    order = list(sites) + [q for q in range(NQ) if q not in sites]
    inv = np.argsort(np.array(order))
    perm = [int(p) for p in inv] + [NQ + int(p) for p in inv]
    return t.transpose(perm).reshape(D, D)


def _build_operators(features, W1, b1, W2, b2, H_self, H_coupling, rates):
    f32 = np.float32
    h = np.maximum(np.asarray(features, f32) @ np.asarray(W1, f32) + np.asarray(b1, f32), 0)
    ops = (h @ np.asarray(W2, f32) + np.asarray(b2, f32)).reshape(NQ, 2, 2)
    Hs = np.asarray(H_self, f32)
    Hc = np.asarray(H_coupling, f32)
    rates = np.asarray(rates, f32)

    H = np.zeros((D, D), f32)
    for i in range(NQ):
        Hi = ops[i] @ Hs[i] + Hs[i].T @ ops[i].T
        H += _embed(Hi, [i])
    for i in range(NQ):
        for j in range(i + 1, NQ):
            oij = np.kron(ops[i], ops[j])
            Hij = oij @ Hc[i, j] + Hc[i, j].T @ oij.T
            H += _embed(Hij, [i, j])

    Ls = []
    for i in range(NQ):
        for j in range(NQ):
            g = np.sqrt(np.abs(rates[i, j])).astype(f32)
            if i == j:
                Ls.append(_embed(g[:2, :2] * ops[i], [i]))
            else:
                Ls.append(_embed(g * np.kron(ops[i], ops[j]), [i, j]))
    L = np.stack(Ls)                                      # (49, D, D) real
    M = np.einsum('kji,kjl->il', L, L, optimize=True)     # sum_k L^T L
    return H, L, M


# ------------------------------------------------------------- device kernel
def _build_nc(dts, repeat=1, strategy="ar", taylor_J=None):
    """One SPMD graph for all 8 cores. Per-core data differences (which L
    ops, whether A/B are nonzero) come via inputs only.

    State X = [P | Q] (real | imag), 128x256. Per stage:
      Fr = A Q - Q A + Bn P + P Bn + sum_k L_k P L_k^T     (Bn = -M/2)
      Fi = -A P + P A + Bn Q + Q Bn + sum_k L_k Q L_k^T
    computed via matmul(out, lhsT, rhs) = lhsT^T @ rhs with
      V_k = (L_k X)^T      <- lhsT=X,   rhs=L_k^T
      L_k X L_k^T          <- lhsT=V_k, rhs=L_k^T  (PSUM accumulate)
    AB terms use the Hermitian structure (P^T = P, Q^T = -Q).
    """
    nsteps = len(dts)
    nc = bacc.Bacc(None, target_bir_lowering=False, debug=False,
                   num_devices=NCORES)
    lt_in = nc.dram_tensor("lt", [D, SLOTS * D], BF16, kind="ExternalInput")
    ab_in = nc.dram_tensor("ab", [D, 4 * D], BF16, kind="ExternalInput")
    x0_in = nc.dram_tensor("x0", [D, 2 * D], F32, kind="ExternalInput")
    traj = nc.dram_tensor("traj", [nsteps, D, 2 * D], F32, kind="ExternalOutput")
    rg = [list(range(NCORES))]

    with tile.TileContext(nc) as tc:
        with (
            tc.tile_pool(name="const", bufs=1) as const,
            tc.tile_pool(name="state", bufs=1) as state,
            tc.tile_pool(name="xb", bufs=2) as xbp,
            tc.tile_pool(name="vsb", bufs=1) as vsb,
            tc.tile_pool(name="pack", bufs=2) as packp,
            tc.tile_pool(name="vps", bufs=1, space="PSUM") as vps,
            tc.tile_pool(name="accps", bufs=1, space="PSUM") as accps,
            tc.tile_pool(name="dram", bufs=2, space="DRAM") as dram,
        ):
            LT = const.tile([D, SLOTS * D], BF16, name="LT")
            AB = const.tile([D, 4 * D], BF16, name="AB")
            nc.sync.dma_start(LT[:], lt_in[:])
            nc.sync.dma_start(AB[:], ab_in[:])

            acc = state.tile([D, 2 * D], F32, name="acc")
            nc.sync.dma_start(acc[:], x0_in[:])

            xb0 = xbp.tile([D, 2 * D], BF16, name="xb0", tag="xb")
            nc.vector.tensor_copy(xb0[:], acc[:])
            Xb = xb0

            def f_stage(it, j, Xb):
                """One application of F: returns the all-reduced next state."""
                P = Xb[:, 0:D]
                Q = Xb[:, D:2 * D]
                A = AB[:, 0:D]
                Bn = AB[:, D:2 * D]
                An = AB[:, 2 * D:3 * D]     # -A
                Bnn = AB[:, 3 * D:4 * D]    # -Bn

                Vp = vps.tile([D, SLOTS * D], F32, name=f"vp{it}_{j}", tag="vp")
                Vq = vps.tile([D, SLOTS * D], F32, name=f"vq{it}_{j}", tag="vq")
                Fr = accps.tile([D, D], F32, name=f"fr{it}_{j}", tag="fr")
                Fip = accps.tile([D, D], F32, name=f"fip{it}_{j}", tag="fip")

                # Fr  = A Q - Q A + Bn P + P Bn + S(P)
                # Fi  = -A P + P A + Bn Q + Q Bn + S(Q)
                # (uses Hermitian structure: P^T = P, Q^T = -Q)
                nc.tensor.matmul(Vp[:, 0:512], lhsT=P, rhs=LT[:, 0:512])
                nc.tensor.matmul(Vp[:, 512:896], lhsT=P, rhs=LT[:, 512:896])
                nc.tensor.matmul(Fr[:], lhsT=P, rhs=Bn, start=True, stop=False)
                nc.tensor.matmul(Fip[:], lhsT=P, rhs=A, start=True, stop=False)
                nc.tensor.matmul(Vq[:, 0:512], lhsT=Q, rhs=LT[:, 0:512])
                nc.tensor.matmul(Vq[:, 512:896], lhsT=Q, rhs=LT[:, 512:896])
                nc.tensor.matmul(Fr[:], lhsT=Q, rhs=A, start=False, stop=False)
                nc.tensor.matmul(Fip[:], lhsT=Q, rhs=Bnn, start=False, stop=False)
                nc.tensor.matmul(Fr[:], lhsT=A, rhs=Q, start=False, stop=False)
                nc.tensor.matmul(Fip[:], lhsT=An, rhs=P, start=False, stop=False)
                nc.tensor.matmul(Fr[:], lhsT=Bn, rhs=P, start=False, stop=False)
                nc.tensor.matmul(Fip[:], lhsT=Bn, rhs=Q, start=False, stop=False)

                Vp_sb = vsb.tile([D, SLOTS * D], BF16, name=f"vps{it}_{j}", tag="vpsb")
                Vq_sb = vsb.tile([D, SLOTS * D], BF16, name=f"vqs{it}_{j}", tag="vqsb")
                nc.vector.tensor_copy(Vp_sb[:, 0:512], Vp[:, 0:512])
                nc.vector.tensor_copy(Vp_sb[:, 512:896], Vp[:, 512:896])
                nc.vector.tensor_copy(Vq_sb[:, 0:512], Vq[:, 0:512])
                nc.vector.tensor_copy(Vq_sb[:, 512:896], Vq[:, 512:896])

                for s in range(SLOTS):
                    sl = slice(s * D, (s + 1) * D)
                    nc.tensor.matmul(Fr[:], lhsT=Vp_sb[:, sl], rhs=LT[:, sl],
                                     start=False, stop=(s == SLOTS - 1))
                    nc.tensor.matmul(Fip[:], lhsT=Vq_sb[:, sl], rhs=LT[:, sl],
                                     start=False, stop=(s == SLOTS - 1))

                part = packp.tile([D, 2 * D], BF16, name=f"pt{it}_{j}", tag="part")
                nc.vector.tensor_copy(part[:, 0:D], Fr[:])
                nc.vector.tensor_copy(part[:, D:2 * D], Fip[:])

                cin = dram.tile([D, 2 * D], BF16, name=f"ci{it}_{j}", tag="cin")
                nc.sync.dma_start(cin[:], part[:])
                Xn = xbp.tile([D, 2 * D], BF16, name=f"xb{it}_{j}", tag="xb")
                if strategy == "ar":
                    cout = dram.tile([D, 2 * D], BF16,
                                     name=f"co{it}_{j}", tag="cout")
                    nc.gpsimd.collective_compute(
                        "AllReduce", AluOp.add, replica_groups=rg,
                        ins=[cin[:].opt()], outs=[cout[:].opt()])
                    nc.sync.dma_start(Xn[:], cout[:])
                else:  # "ag": AllGather + local tree-sum
                    gout = dram.tile([NCORES * D, 2 * D], BF16,
                                     name=f"go{it}_{j}", tag="gout")
                    nc.gpsimd.collective_compute(
                        "AllGather", AluOp.bypass, replica_groups=rg,
                        ins=[cin[:].opt()], outs=[gout[:].opt()])
                    gsb = packp.tile([D, NCORES * 2 * D], BF16,
                                     name=f"gs{it}_{j}", tag="gsb")
                    for g in range(NCORES):
                        nc.sync.dma_start(
                            gsb[:, g * 256:(g + 1) * 256],
                            gout[g * D:(g + 1) * D, :])
                    t4 = packp.tile([D, 4 * 2 * D], BF16,
                                    name=f"t4{it}_{j}", tag="t4")
                    nc.vector.tensor_tensor(t4[:], gsb[:, 0:1024],
                                            gsb[:, 1024:2048], op=AluOp.add)
                    t2 = packp.tile([D, 2 * 2 * D], BF16,
                                    name=f"t2{it}_{j}", tag="t2")
                    nc.vector.tensor_tensor(t2[:], t4[:, 0:512],
                                            t4[:, 512:1024], op=AluOp.add)
                    nc.vector.tensor_tensor(Xn[:], t2[:, 0:256],
                                            t2[:, 256:512], op=AluOp.add)
                return Xn

            if taylor_J is not None:
                # Single Taylor chain w_j = F^j(rho0); per-output coefficients
                # (tau_t)^j / j! with tau_t = t_eval[t+1] - t_eval[0].
                import math as _math
                taus = [float(sum(dts[:tt + 1])) for tt in range(nsteps)]
                accs = []
                for tt in range(nsteps):
                    a = state.tile([D, 2 * D], F32, name=f"acc{tt}")
                    nc.sync.dma_start(a[:], x0_in[:])
                    accs.append(a)
                for rrep in range(repeat):
                    Xc = Xb
                    for j in range(1, taylor_J + 1):
                        Xc = f_stage(rrep, j, Xc)
                        for tt in range(nsteps):
                            c = taus[tt] ** j / _math.factorial(j)
                            nc.vector.scalar_tensor_tensor(
                                accs[tt][:], Xc[:], c, accs[tt][:],
                                op0=AluOp.mult, op1=AluOp.add)
                for tt in range(nsteps):
                    nc.sync.dma_start(traj[tt, :, :], accs[tt][:])
            else:
                for it, t in enumerate(
                        [s for _ in range(repeat) for s in range(nsteps)]):
                    dt = float(dts[t])
                    cs = [dt, dt * dt / 2.0, dt ** 3 / 6.0, dt ** 4 / 24.0]
                    for j in range(4):
                        Xn = f_stage(it, j, Xb)
                        nc.vector.scalar_tensor_tensor(
                            acc[:], Xn[:], cs[j], acc[:],
                            op0=AluOp.mult, op1=AluOp.add)
                        Xb = Xn
                    nc.sync.dma_start(traj[t, :, :], acc[:])
                    if it + 1 < nsteps * repeat:
                        xs = xbp.tile([D, 2 * D], BF16, name=f"xs{it}", tag="xb")
                        nc.vector.tensor_copy(xs[:], acc[:])
                        Xb = xs
    nc.compile()
    return nc


# ---------------------------------------------------------------- jit runner
class _Runner:
    """Persistent jitted shard_map executor for a compiled Bass graph
    (mirrors bass2jax.run_bass_via_pjrt, but reusable for timing)."""

    def __init__(self, nc):
        import jax
        from jax.sharding import Mesh, PartitionSpec
        from jax.experimental.shard_map import shard_map
        from concourse import bass2jax
        bass2jax.install_neuronx_cc_hook()

        self.nc = nc
        part_name = nc.partition_id_tensor.name if nc.partition_id_tensor else None
        in_names, out_names, out_avals, zero_outs = [], [], [], []
        for alloc in nc.m.functions[0].allocations:
            if not isinstance(alloc, mybir.MemoryLocationSet):
                continue
            name = alloc.memorylocations[0].name
            if alloc.kind == "ExternalInput":
                if name != part_name:
                    in_names.append(name)
            elif alloc.kind == "ExternalOutput":
                out_names.append(name)
                shape = tuple(alloc.tensor_shape)
                dtype = mybir.dt.np(alloc.dtype)
                out_avals.append(jax.core.ShapedArray(shape, dtype))
                zero_outs.append(np.zeros(shape, dtype))
        self.in_names, self.out_names = in_names, out_names
        self.out_avals, self.zero_outs = out_avals, zero_outs
        n_params, n_outs = len(in_names), len(out_names)

        def _body(*args):
            operands = list(args)
            bind_names = in_names + out_names
            if part_name is not None:
                operands.append(bass2jax.partition_id_tensor())
                bind_names = bind_names + [part_name]
            outs = bass2jax._bass_exec_p.bind(
                *operands,
                out_avals=tuple(out_avals),
                in_names=tuple(bind_names),
                out_names=tuple(out_names),
                lowering_input_output_aliases=(),
                sim_require_finite=True,
                sim_require_nnan=True,
                nc=nc,
            )
            return tuple(outs)

        devices = jax.devices()[:NCORES]
        self.mesh = Mesh(np.asarray(devices), ("core",))
        specs = (PartitionSpec("core"),) * (n_params + n_outs)
        self.fn = jax.jit(
            shard_map(_body, mesh=self.mesh, in_specs=specs,
                      out_specs=(PartitionSpec("core"),) * n_outs,
                      check_rep=False),
            donate_argnums=tuple(range(n_params, n_params + n_outs)),
            keep_unused=True,
        )
        self.jax = jax

    def _concat_inputs(self, in_maps):
        return [np.concatenate([np.asarray(in_maps[c][n]) for c in range(NCORES)],
                               axis=0) for n in self.in_names]

    def _zeros(self):
        return [np.zeros((NCORES * z.shape[0], *z.shape[1:]), z.dtype)
                for z in self.zero_outs]

    def run(self, in_maps):
        outs = self.fn(*self._concat_inputs(in_maps), *self._zeros())
        return {
            n: np.asarray(outs[i]).reshape(NCORES, *self.out_avals[i].shape)
            for i, n in enumerate(self.out_names)
        }

    def time(self, in_maps, reps=30):
        """Median-of-batches pipelined timing: returns est seconds/execution."""
        import time as _time
        cin = [self.jax.device_put(x) for x in self._concat_inputs(in_maps)]
        zsets = [[self.jax.device_put(z) for z in self._zeros()]
                 for _ in range(reps)]
        self.jax.block_until_ready((cin, zsets))
        outs = self.fn(*cin, *zsets[0])          # warm
        self.jax.block_until_ready(outs)
        t0 = _time.time()
        res = [self.fn(*cin, *z) for z in zsets[1:]]
        self.jax.block_until_ready(res)
        t1 = _time.time()
        return (t1 - t0) / max(1, reps - 1)


# -------------------------------------------------------------------- driver
_CACHE = {}


def _get_runner(dts, taylor_J=None):
    key = (tuple(np.float32(d) for d in dts), taylor_J)
    if key not in _CACHE:
        _CACHE[key] = _Runner(_build_nc([float(d) for d in key[0]],
                                        taylor_J=taylor_J))
    return _CACHE[key]


def _pick_taylor_J(H, L, M, dts):
    """Host-side convergence check for the direct Taylor evaluation
    rho(tau_t) = sum_j tau_t^j/j! F^j(rho0). Returns J (number of device
    F-stages) if the series converges fast enough AND the reference's RK4
    is itself within ~1e-6 of the exact exponential; else None (use the
    exact RK4-replication path)."""
    import math
    A, Bn = H, -0.5 * M
    taus = np.cumsum(np.asarray(dts, np.float64))

    def Fm(P, Q):
        LP = L @ P
        SP = np.einsum('kij,kmj->im', LP, L, optimize=True)
        LQ = L @ Q
        SQ = np.einsum('kij,kmj->im', LQ, L, optimize=True)
        return (A @ Q - Q @ A + Bn @ P + P @ Bn + SP,
                -A @ P + P @ A + Bn @ Q + Q @ Bn + SQ)

    P = np.eye(D, dtype=np.float32)
    Q = np.zeros_like(P)
    tmax = float(taus[-1])
    scale = np.linalg.norm(P)
    wn = [scale]
    for j in range(1, 17):
        P, Q = Fm(P, Q)
        wn.append(float(np.hypot(np.linalg.norm(P), np.linalg.norm(Q))))
        tail = tmax ** j / math.factorial(j) * wn[j]
        if j >= 3 and tail < 1e-8 * scale:
            # RK4-vs-exp mismatch bound: per step ~ dt^5/120 * |F^5(rho)|
            dt5 = max(float(d) for d in dts) ** 5
            if j >= 5:
                rk4_gap = dt5 / 120.0 * wn[5] * len(dts)
            else:
                rk4_gap = 0.0
            if rk4_gap < 1e-6 * scale:
                return j
            return None
    return None


def _in_maps(H, L, M, P0, Q0):
    bf = ml_dtypes.bfloat16
    Bn = (-0.5 * M).astype(np.float32)
    ab0 = np.concatenate([H, Bn, -H, -Bn], axis=1).astype(bf)
    x0 = np.concatenate([np.asarray(P0, np.float32),
                         np.asarray(Q0, np.float32)], axis=1)
    maps, k0 = [], 0
    for c in range(NCORES):
        n = OP_SPLIT[c]
        lt = np.zeros((D, SLOTS * D), np.float32)
        for s in range(n):
            lt[:, s * D:(s + 1) * D] = L[k0 + s].T
        k0 += n
        maps.append({
            "lt": lt.astype(bf),
            "ab": ab0 if c == 0 else np.zeros_like(ab0),
            "x0": x0,
        })
    return maps


def _solve(runner, H, L, M, P0, Q0, nsteps):
    res = runner.run(_in_maps(H, L, M, P0, Q0))
    tr = res["traj"][0]          # all cores identical; [nsteps, 128, 256]
    out = np.empty((nsteps + 1, D, D), np.complex64)
    out[0] = np.asarray(P0, np.float32) + 1j * np.asarray(Q0, np.float32)
    for t in range(nsteps):
        out[t + 1] = tr[t, :, 0:D] + 1j * tr[t, :, D:2 * D]
    return out


def kernel(features, t_eval, W1, b1, W2, b2, H_self, H_coupling,
           lindblad_rates, rho_0):
    H, L, M = _build_operators(features, W1, b1, W2, b2,
                               H_self, H_coupling, lindblad_rates)
    t_eval = np.asarray(t_eval, np.float32)
    dts = (t_eval[1:] - t_eval[:-1]).astype(np.float32)
    taylor_J = _pick_taylor_J(H, L, M, dts)
    runner = _get_runner(dts, taylor_J)

    rho0 = np.asarray(rho_0, np.float32)
    sym = np.abs(rho0 - rho0.T).max() <= 1e-6 * max(1.0, np.abs(rho0).max())
    if sym:
        return _solve(runner, H, L, M, rho0, np.zeros_like(rho0), len(dts))
    # non-Hermitian rho_0: split into Hermitian parts and run twice
    S = 0.5 * (rho0 + rho0.T)
    K = 0.5 * (rho0 - rho0.T)
    tA = _solve(runner, H, L, M, S, np.zeros_like(S), len(dts))
    tB = _solve(runner, H, L, M, np.zeros_like(K), K, len(dts))
    return (tA + (-1j) * tB).astype(np.complex64)


# revision 20
# speedup vs baseline: 250.5844x; 250.5844x over previous
"""Trainium2 Bass kernel for the DanceDynamicsModel Lindblad solver.

Full inputs in, full outputs out. Internally:
  - host (numpy): build the 128x128 Hamiltonian H, the 49 Lindblad
    operators L_k, and M = sum_k L_k^T L_k from the tiny MLP inputs.
  - device (8 NeuronCores): the linear Lindblad map F applied as a chain,
    with the 49 L-sandwich terms sharded over cores and an AllReduce per
    stage. Two modes:
      * Taylor-direct (default when the host-side convergence check
        passes): one chain w_j = F^j(rho0), all T outputs formed as
        rho(tau_t) = sum_j tau_t^j/j! w_j.  J is chosen adaptively
        (J=3 for the reference inputs -> 3 device stages total).
      * exact RK4 replication (fallback): 4 stages per step, matching
        the reference integrator term by term.

All device matmuls are bf16 (PSUM accumulates fp32); validated at
~8e-7 global relative error vs the complex64 reference.
"""
import sys
for _p in ('/opt/trn_rl_repo',):
    if _p not in sys.path:
        sys.path.insert(0, _p)

import numpy as np
import ml_dtypes

import concourse.bass as bass
import concourse.bacc as bacc
import concourse.tile as tile
import concourse.mybir as mybir

NQ = 7          # qubits ("dancers")
D = 128         # 2**NQ
NCORES = 8
SLOTS = 7       # Lindblad-op slots per core (49 real ops + AB task, padded)
# ops per core; core 0 also owns the Hamiltonian/M ("AB") terms
OP_SPLIT = [4, 7, 7, 7, 6, 6, 6, 6]
BF16 = mybir.dt.bfloat16
F32 = mybir.dt.float32
AluOp = mybir.AluOpType


# ----------------------------------------------------------------- host math
def _embed(op, sites):
    k = len(sites)
    full = np.kron(op, np.eye(2 ** (NQ - k), dtype=op.dtype))
    t = full.reshape((2,) * (2 * NQ))
    order = list(sites) + [q for q in range(NQ) if q not in sites]
    inv = np.argsort(np.array(order))
    perm = [int(p) for p in inv] + [NQ + int(p) for p in inv]
    return t.transpose(perm).reshape(D, D)


def _build_operators(features, W1, b1, W2, b2, H_self, H_coupling, rates):
    f32 = np.float32
    h = np.maximum(np.asarray(features, f32) @ np.asarray(W1, f32) + np.asarray(b1, f32), 0)
    ops = (h @ np.asarray(W2, f32) + np.asarray(b2, f32)).reshape(NQ, 2, 2)
    Hs = np.asarray(H_self, f32)
    Hc = np.asarray(H_coupling, f32)
    rates = np.asarray(rates, f32)

    H = np.zeros((D, D), f32)
    for i in range(NQ):
        Hi = ops[i] @ Hs[i] + Hs[i].T @ ops[i].T
        H += _embed(Hi, [i])
    for i in range(NQ):
        for j in range(i + 1, NQ):
            oij = np.kron(ops[i], ops[j])
            Hij = oij @ Hc[i, j] + Hc[i, j].T @ oij.T
            H += _embed(Hij, [i, j])

    Ls = []
    for i in range(NQ):
        for j in range(NQ):
            g = np.sqrt(np.abs(rates[i, j])).astype(f32)
            if i == j:
                Ls.append(_embed(g[:2, :2] * ops[i], [i]))
            else:
                Ls.append(_embed(g * np.kron(ops[i], ops[j]), [i, j]))
    L = np.stack(Ls)                                      # (49, D, D) real
    M = np.einsum('kji,kjl->il', L, L, optimize=True)     # sum_k L^T L
    return H, L, M


# ------------------------------------------------------------- device kernel
def _build_nc(dts, repeat=1, strategy="ar", taylor_J=None):
    """One SPMD graph for all 8 cores. Per-core data differences (which L
    ops, whether A/B are nonzero) come via inputs only.

    State X = [P | Q] (real | imag), 128x256. Per stage:
      Fr = A Q - Q A + Bn P + P Bn + sum_k L_k P L_k^T     (Bn = -M/2)
      Fi = -A P + P A + Bn Q + Q Bn + sum_k L_k Q L_k^T
    computed via matmul(out, lhsT, rhs) = lhsT^T @ rhs with
      V_k = (L_k X)^T      <- lhsT=X,   rhs=L_k^T
      L_k X L_k^T          <- lhsT=V_k, rhs=L_k^T  (PSUM accumulate)
    AB terms use the Hermitian structure (P^T = P, Q^T = -Q).
    """
    nsteps = len(dts)
    nc = bacc.Bacc(None, target_bir_lowering=False, debug=False,
                   num_devices=NCORES)
    lt_in = nc.dram_tensor("lt", [D, SLOTS * D], BF16, kind="ExternalInput")
    ab_in = nc.dram_tensor("ab", [D, 4 * D], BF16, kind="ExternalInput")
    x0_in = nc.dram_tensor("x0", [D, 2 * D], F32, kind="ExternalInput")
    traj = nc.dram_tensor("traj", [nsteps, D, 2 * D], F32, kind="ExternalOutput")
    rg = [list(range(NCORES))]

    with tile.TileContext(nc) as tc:
        with (
            tc.tile_pool(name="const", bufs=1) as const,
            tc.tile_pool(name="state", bufs=1) as state,
            tc.tile_pool(name="xb", bufs=2) as xbp,
            tc.tile_pool(name="vsb", bufs=1) as vsb,
            tc.tile_pool(name="pack", bufs=2) as packp,
            tc.tile_pool(name="vps", bufs=1, space="PSUM") as vps,
            tc.tile_pool(name="accps", bufs=1, space="PSUM") as accps,
            tc.tile_pool(name="dram", bufs=2, space="DRAM") as dram,
        ):
            LT = const.tile([D, SLOTS * D], BF16, name="LT")
            AB = const.tile([D, 4 * D], BF16, name="AB")
            nc.sync.dma_start(LT[:], lt_in[:])
            nc.sync.dma_start(AB[:], ab_in[:])

            acc = state.tile([D, 2 * D], F32, name="acc")
            nc.sync.dma_start(acc[:], x0_in[:])

            xb0 = xbp.tile([D, 2 * D], BF16, name="xb0", tag="xb")
            nc.vector.tensor_copy(xb0[:], acc[:])
            Xb = xb0

            def f_stage(it, j, Xb):
                """One application of F: returns the all-reduced next state."""
                P = Xb[:, 0:D]
                Q = Xb[:, D:2 * D]
                A = AB[:, 0:D]
                Bn = AB[:, D:2 * D]
                An = AB[:, 2 * D:3 * D]     # -A
                Bnn = AB[:, 3 * D:4 * D]    # -Bn

                Vp = vps.tile([D, SLOTS * D], F32, name=f"vp{it}_{j}", tag="vp")
                Vq = vps.tile([D, SLOTS * D], F32, name=f"vq{it}_{j}", tag="vq")
                Fr = accps.tile([D, D], F32, name=f"fr{it}_{j}", tag="fr")
                Fip = accps.tile([D, D], F32, name=f"fip{it}_{j}", tag="fip")

                # Fr  = A Q - Q A + Bn P + P Bn + S(P)
                # Fi  = -A P + P A + Bn Q + Q Bn + S(Q)
                # (uses Hermitian structure: P^T = P, Q^T = -Q)
                nc.tensor.matmul(Vp[:, 0:512], lhsT=P, rhs=LT[:, 0:512])
                nc.tensor.matmul(Vp[:, 512:896], lhsT=P, rhs=LT[:, 512:896])
                nc.tensor.matmul(Fr[:], lhsT=P, rhs=Bn, start=True, stop=False)
                nc.tensor.matmul(Fip[:], lhsT=P, rhs=A, start=True, stop=False)
                nc.tensor.matmul(Vq[:, 0:512], lhsT=Q, rhs=LT[:, 0:512])
                nc.tensor.matmul(Vq[:, 512:896], lhsT=Q, rhs=LT[:, 512:896])
                nc.tensor.matmul(Fr[:], lhsT=Q, rhs=A, start=False, stop=False)
                nc.tensor.matmul(Fip[:], lhsT=Q, rhs=Bnn, start=False, stop=False)
                nc.tensor.matmul(Fr[:], lhsT=A, rhs=Q, start=False, stop=False)
                nc.tensor.matmul(Fip[:], lhsT=An, rhs=P, start=False, stop=False)
                nc.tensor.matmul(Fr[:], lhsT=Bn, rhs=P, start=False, stop=False)
                nc.tensor.matmul(Fip[:], lhsT=Bn, rhs=Q, start=False, stop=False)

                Vp_sb = vsb.tile([D, SLOTS * D], BF16, name=f"vps{it}_{j}", tag="vpsb")
                Vq_sb = vsb.tile([D, SLOTS * D], BF16, name=f"vqs{it}_{j}", tag="vqsb")
                nc.vector.tensor_copy(Vp_sb[:, 0:512], Vp[:, 0:512])
                nc.vector.tensor_copy(Vp_sb[:, 512:896], Vp[:, 512:896])
                nc.vector.tensor_copy(Vq_sb[:, 0:512], Vq[:, 0:512])
                nc.vector.tensor_copy(Vq_sb[:, 512:896], Vq[:, 512:896])

                for s in range(SLOTS):
                    sl = slice(s * D, (s + 1) * D)
                    nc.tensor.matmul(Fr[:], lhsT=Vp_sb[:, sl], rhs=LT[:, sl],
                                     start=False, stop=(s == SLOTS - 1))
                    nc.tensor.matmul(Fip[:], lhsT=Vq_sb[:, sl], rhs=LT[:, sl],
                                     start=False, stop=(s == SLOTS - 1))

                part = packp.tile([D, 2 * D], BF16, name=f"pt{it}_{j}", tag="part")
                nc.vector.tensor_copy(part[:, 0:D], Fr[:])
                nc.vector.tensor_copy(part[:, D:2 * D], Fip[:])

                cin = dram.tile([D, 2 * D], BF16, name=f"ci{it}_{j}", tag="cin")
                nc.sync.dma_start(cin[:], part[:])
                Xn = xbp.tile([D, 2 * D], BF16, name=f"xb{it}_{j}", tag="xb")
                if strategy == "ar":
                    cout = dram.tile([D, 2 * D], BF16,
                                     name=f"co{it}_{j}", tag="cout")
                    nc.gpsimd.collective_compute(
                        "AllReduce", AluOp.add, replica_groups=rg,
                        ins=[cin[:].opt()], outs=[cout[:].opt()])
                    nc.sync.dma_start(Xn[:], cout[:])
                else:  # "ag": AllGather + local tree-sum
                    gout = dram.tile([NCORES * D, 2 * D], BF16,
                                     name=f"go{it}_{j}", tag="gout")
                    nc.gpsimd.collective_compute(
                        "AllGather", AluOp.bypass, replica_groups=rg,
                        ins=[cin[:].opt()], outs=[gout[:].opt()])
                    gsb = packp.tile([D, NCORES * 2 * D], BF16,
                                     name=f"gs{it}_{j}", tag="gsb")
                    for g in range(NCORES):
                        nc.sync.dma_start(
                            gsb[:, g * 256:(g + 1) * 256],
                            gout[g * D:(g + 1) * D, :])
                    t4 = packp.tile([D, 4 * 2 * D], BF16,
                                    name=f"t4{it}_{j}", tag="t4")
                    nc.vector.tensor_tensor(t4[:], gsb[:, 0:1024],
                                            gsb[:, 1024:2048], op=AluOp.add)
                    t2 = packp.tile([D, 2 * 2 * D], BF16,
                                    name=f"t2{it}_{j}", tag="t2")
                    nc.vector.tensor_tensor(t2[:], t4[:, 0:512],
                                            t4[:, 512:1024], op=AluOp.add)
                    nc.vector.tensor_tensor(Xn[:], t2[:, 0:256],
                                            t2[:, 256:512], op=AluOp.add)
                return Xn

            if taylor_J is not None:
                # Single Taylor chain w_j = F^j(rho0); per-output coefficients
                # (tau_t)^j / j! with tau_t = t_eval[t+1] - t_eval[0].
                import math as _math
                taus = [float(sum(dts[:tt + 1])) for tt in range(nsteps)]
                accs = []
                for tt in range(nsteps):
                    a = state.tile([D, 2 * D], F32, name=f"acc{tt}")
                    nc.sync.dma_start(a[:], x0_in[:])
                    accs.append(a)
                for rrep in range(repeat):
                    Xc = Xb
                    for j in range(1, taylor_J + 1):
                        Xc = f_stage(rrep, j, Xc)
                        for tt in range(nsteps):
                            c = taus[tt] ** j / _math.factorial(j)
                            nc.vector.scalar_tensor_tensor(
                                accs[tt][:], Xc[:], c, accs[tt][:],
                                op0=AluOp.mult, op1=AluOp.add)
                for tt in range(nsteps):
                    nc.sync.dma_start(traj[tt, :, :], accs[tt][:])
            else:
                for it, t in enumerate(
                        [s for _ in range(repeat) for s in range(nsteps)]):
                    dt = float(dts[t])
                    cs = [dt, dt * dt / 2.0, dt ** 3 / 6.0, dt ** 4 / 24.0]
                    for j in range(4):
                        Xn = f_stage(it, j, Xb)
                        nc.vector.scalar_tensor_tensor(
                            acc[:], Xn[:], cs[j], acc[:],
                            op0=AluOp.mult, op1=AluOp.add)
                        Xb = Xn
                    nc.sync.dma_start(traj[t, :, :], acc[:])
                    if it + 1 < nsteps * repeat:
                        xs = xbp.tile([D, 2 * D], BF16, name=f"xs{it}", tag="xb")
                        nc.vector.tensor_copy(xs[:], acc[:])
                        Xb = xs
    nc.compile()
    return nc


# ---------------------------------------------------------------- jit runner
class _Runner:
    """Persistent jitted shard_map executor for a compiled Bass graph
    (mirrors bass2jax.run_bass_via_pjrt, but reusable for timing)."""

    def __init__(self, nc):
        import jax
        from jax.sharding import Mesh, PartitionSpec
        from jax.experimental.shard_map import shard_map
        from concourse import bass2jax
        bass2jax.install_neuronx_cc_hook()

        self.nc = nc
        part_name = nc.partition_id_tensor.name if nc.partition_id_tensor else None
        in_names, out_names, out_avals, zero_outs = [], [], [], []
        for alloc in nc.m.functions[0].allocations:
            if not isinstance(alloc, mybir.MemoryLocationSet):
                continue
            name = alloc.memorylocations[0].name
            if alloc.kind == "ExternalInput":
                if name != part_name:
                    in_names.append(name)
            elif alloc.kind == "ExternalOutput":
                out_names.append(name)
                shape = tuple(alloc.tensor_shape)
                dtype = mybir.dt.np(alloc.dtype)
                out_avals.append(jax.core.ShapedArray(shape, dtype))
                zero_outs.append(np.zeros(shape, dtype))
        self.in_names, self.out_names = in_names, out_names
        self.out_avals, self.zero_outs = out_avals, zero_outs
        n_params, n_outs = len(in_names), len(out_names)

        def _body(*args):
            operands = list(args)
            bind_names = in_names + out_names
            if part_name is not None:
                operands.append(bass2jax.partition_id_tensor())
                bind_names = bind_names + [part_name]
            outs = bass2jax._bass_exec_p.bind(
                *operands,
                out_avals=tuple(out_avals),
                in_names=tuple(bind_names),
                out_names=tuple(out_names),
                lowering_input_output_aliases=(),
                sim_require_finite=True,
                sim_require_nnan=True,
                nc=nc,
            )
            return tuple(outs)

        devices = jax.devices()[:NCORES]
        self.mesh = Mesh(np.asarray(devices), ("core",))
        specs = (PartitionSpec("core"),) * (n_params + n_outs)
        self.fn = jax.jit(
            shard_map(_body, mesh=self.mesh, in_specs=specs,
                      out_specs=(PartitionSpec("core"),) * n_outs,
                      check_rep=False),
            donate_argnums=tuple(range(n_params, n_params + n_outs)),
            keep_unused=True,
        )
        self.jax = jax

    def _concat_inputs(self, in_maps):
        return [np.concatenate([np.asarray(in_maps[c][n]) for c in range(NCORES)],
                               axis=0) for n in self.in_names]

    def _zeros(self):
        return [np.zeros((NCORES * z.shape[0], *z.shape[1:]), z.dtype)
                for z in self.zero_outs]

    def run(self, in_maps):
        outs = self.fn(*self._concat_inputs(in_maps), *self._zeros())
        return {
            n: np.asarray(outs[i]).reshape(NCORES, *self.out_avals[i].shape)
            for i, n in enumerate(self.out_names)
        }

    def time(self, in_maps, reps=30):
        """Median-of-batches pipelined timing: returns est seconds/execution."""
        import time as _time
        cin = [self.jax.device_put(x) for x in self._concat_inputs(in_maps)]
        zsets = [[self.jax.device_put(z) for z in self._zeros()]
                 for _ in range(reps)]
        self.jax.block_until_ready((cin, zsets))
        outs = self.fn(*cin, *zsets[0])          # warm
        self.jax.block_until_ready(outs)
        t0 = _time.time()
        res = [self.fn(*cin, *z) for z in zsets[1:]]
        self.jax.block_until_ready(res)
        t1 = _time.time()
        return (t1 - t0) / max(1, reps - 1)


# -------------------------------------------------------------------- driver
_CACHE = {}


def _get_runner(dts, taylor_J=None):
    key = (tuple(np.float32(d) for d in dts), taylor_J)
    if key not in _CACHE:
        _CACHE[key] = _Runner(_build_nc([float(d) for d in key[0]],
                                        taylor_J=taylor_J))
    return _CACHE[key]


def _pick_taylor_J(H, L, M, dts):
    """Host-side convergence check for the direct Taylor evaluation
    rho(tau_t) = sum_j tau_t^j/j! F^j(rho0). Returns J (number of device
    F-stages) if the series converges fast enough AND the reference's RK4
    is itself within ~1e-6 of the exact exponential; else None (use the
    exact RK4-replication path)."""
    import math
    A, Bn = H, -0.5 * M
    taus = np.cumsum(np.asarray(dts, np.float64))

    def Fm(P, Q):
        LP = L @ P
        SP = np.einsum('kij,kmj->im', LP, L, optimize=True)
        LQ = L @ Q
        SQ = np.einsum('kij,kmj->im', LQ, L, optimize=True)
        return (A @ Q - Q @ A + Bn @ P + P @ Bn + SP,
                -A @ P + P @ A + Bn @ Q + Q @ Bn + SQ)

    P = np.eye(D, dtype=np.float32)
    Q = np.zeros_like(P)
    tmax = float(taus[-1])
    scale = np.linalg.norm(P)
    wn = [scale]
    # always compute 5 powers so the RK4-vs-exp gap bound below is informed
    for j in range(1, 17):
        P, Q = Fm(P, Q)
        wn.append(float(np.hypot(np.linalg.norm(P), np.linalg.norm(Q))))
        tail = tmax ** j / math.factorial(j) * wn[j]
        if j >= 5 and tail < 1e-8 * scale:
            # reference RK4 differs from exp by ~ dt^5/120 |F^5(rho)| per step
            dt5 = max(float(d) for d in dts) ** 5
            rk4_gap = dt5 / 120.0 * wn[5] * len(dts)
            if rk4_gap < 1e-6 * scale:
                # smallest J whose truncation tail is below threshold
                for jj in range(3, j + 1):
                    if tmax ** jj / math.factorial(jj) * wn[jj] < 1e-8 * scale:
                        return jj
            return None
    return None


def _in_maps(H, L, M, P0, Q0):
    bf = ml_dtypes.bfloat16
    Bn = (-0.5 * M).astype(np.float32)
    ab0 = np.concatenate([H, Bn, -H, -Bn], axis=1).astype(bf)
    x0 = np.concatenate([np.asarray(P0, np.float32),
                         np.asarray(Q0, np.float32)], axis=1)
    maps, k0 = [], 0
    for c in range(NCORES):
        n = OP_SPLIT[c]
        lt = np.zeros((D, SLOTS * D), np.float32)
        for s in range(n):
            lt[:, s * D:(s + 1) * D] = L[k0 + s].T
        k0 += n
        maps.append({
            "lt": lt.astype(bf),
            "ab": ab0 if c == 0 else np.zeros_like(ab0),
            "x0": x0,
        })
    return maps


def _solve(runner, H, L, M, P0, Q0, nsteps):
    res = runner.run(_in_maps(H, L, M, P0, Q0))
    tr = res["traj"][0]          # all cores identical; [nsteps, 128, 256]
    out = np.empty((nsteps + 1, D, D), np.complex64)
    out[0] = np.asarray(P0, np.float32) + 1j * np.asarray(Q0, np.float32)
    for t in range(nsteps):
        out[t + 1] = tr[t, :, 0:D] + 1j * tr[t, :, D:2 * D]
    return out


def kernel(features, t_eval, W1, b1, W2, b2, H_self, H_coupling,
           lindblad_rates, rho_0):
    H, L, M = _build_operators(features, W1, b1, W2, b2,
                               H_self, H_coupling, lindblad_rates)
    t_eval = np.asarray(t_eval, np.float32)
    dts = (t_eval[1:] - t_eval[:-1]).astype(np.float32)
    taylor_J = _pick_taylor_J(H, L, M, dts)
    runner = _get_runner(dts, taylor_J)

    rho0 = np.asarray(rho_0, np.float32)
    sym = np.abs(rho0 - rho0.T).max() <= 1e-6 * max(1.0, np.abs(rho0).max())
    if sym:
        return _solve(runner, H, L, M, rho0, np.zeros_like(rho0), len(dts))
    # non-Hermitian rho_0: split into Hermitian parts and run twice
    S = 0.5 * (rho0 + rho0.T)
    K = 0.5 * (rho0 - rho0.T)
    tA = _solve(runner, H, L, M, S, np.zeros_like(S), len(dts))
    tB = _solve(runner, H, L, M, np.zeros_like(K), K, len(dts))
    return (tA + (-1j) * tB).astype(np.complex64)
